# revision 10
# baseline (speedup 1.0000x reference)
# Trainium2 Bass kernel for nn_CustomAttention (fused qkv + LoRA + per-head
# LayerNorm + softmax attention + output projection).
#
# Sharding: 16 heads split across 8 cores (2 heads/core), both batch elements
# on every core. Each core computes its heads' attention and its partial
# output projection (sum over its heads' columns); the host sums the 8
# partials and adds proj_b. LoRA is folded into the qkv weights on the host:
#   x@W.T + (x@A)@B*s == x@(W + s*(A@B).T).T
#
# Per-core layout choices (see comments in _build_program):
#  - scores are computed transposed (sT[j,i]) so softmax-normalized output
#    comes out in [d, i] layout, which is exactly the lhsT the projection
#    matmul needs -> no attention-weight transposes at all.
#  - exp without max subtraction (softmax is shift-invariant; post-LayerNorm
#    scores are bounded by ~|D|^0.5 so fp32 exp cannot overflow).
#  - the attention@v matmul uses stationary [v | ones]: output partitions
#    0-63 hold out^T, partitions 64-127 hold the softmax denominator
#    replicated, so normalization is a reciprocal + one multiply.
import numpy as np
import ml_dtypes

import concourse.bass as bass
import concourse.bacc as bacc
import concourse.mybir as mybir
from concourse.tile import TileContext
from concourse.masks import make_identity
from concourse.bass_utils import run_bass_kernel_spmd

BF16 = ml_dtypes.bfloat16
F32 = np.float32

B, N, DIM, H, R = 2, 2048, 1024, 16, 8
D = DIM // H              # 64
NCORES = 8
HPC = H // NCORES         # 2 heads per core
ALPHA = 8.0
LORA_SCALE = ALPHA / R
EPS = 1e-5
QSCALE = float(D) ** -0.5  # 0.125

NCH = DIM // 128          # 8 contraction chunks of 128
NTI = N // 128            # 16 row tiles of 128
NTI8 = N // 256           # 8 i-tiles of 256
QI = 256                  # query-tile width (free dim of score matmuls)

_prog_cache: dict = {}


def _build_program(use_mask: bool, affine_q: bool, affine_k: bool):
    nc = bacc.Bacc("TRN2", target_bir_lowering=False)
    f32 = mybir.dt.float32
    bf16 = mybir.dt.bfloat16

    xT = nc.dram_tensor("xT", [B, NCH, 128, N], bf16, kind="ExternalInput")
    wT = nc.dram_tensor("wT", [NCH, 128, 6 * D], bf16, kind="ExternalInput")
    projT = nc.dram_tensor("projT", [D, HPC, DIM], bf16, kind="ExternalInput")
    out_p = nc.dram_tensor("out_p", [B, N, DIM], f32, kind="ExternalOutput")
    if affine_q or affine_k:
        # rows: 0=qw*scale 1=qb*scale 2=kw 3=kb, each broadcast to 128 parts
        lnaff = nc.dram_tensor("lnaff", [4, 128, D], f32, kind="ExternalInput")
    if use_mask:
        emaskT = nc.dram_tensor("emaskT", [N, N], bf16, kind="ExternalInput")

    with TileContext(nc) as tc:
        import contextlib
        with contextlib.ExitStack() as ctx:
            const = ctx.enter_context(tc.tile_pool(name="const", bufs=1))
            ident = const.tile([128, 128], bf16)
            make_identity(nc, ident)
            eps_t = const.tile([128, 1], f32)
            nc.vector.memset(eps_t, EPS)

            persist = ctx.enter_context(tc.tile_pool(name="persist", bufs=1))
            w_sb = persist.tile([128, NCH, 6 * D], bf16)
            nc.sync.dma_start(out=w_sb, in_=wT.rearrange("ci cm w -> cm ci w"))
            proj_sb = persist.tile([D, HPC, DIM], bf16)
            nc.sync.dma_start(out=proj_sb, in_=projT[:, :, :])
            if affine_q or affine_k:
                aff_sb = persist.tile([128, 4, D], f32)
                nc.sync.dma_start(out=aff_sb, in_=lnaff.rearrange("r p d -> p r d"))

            # per-b persistent activations (rewritten each b; Tile handles WAR)
            xpool = ctx.enter_context(tc.tile_pool(name="xpool", bufs=1))
            qkpool = ctx.enter_context(tc.tile_pool(name="qkpool", bufs=1))
            vpool = ctx.enter_context(tc.tile_pool(name="vpool", bufs=1))

            for b in range(B):
                x_sb = xpool.tile([128, NCH, N], bf16, tag="x_sb")
                nc.sync.dma_start(out=x_sb, in_=xT[b].rearrange("ci cm n -> cm ci n"))
                # qT/kT: partitions 0-63 head0, 64-127 head1; free = n
                qT_sb = qkpool.tile([128, N], bf16, tag="qT")
                kT_sb = qkpool.tile([128, N], bf16, tag="kT")
                # vplus: [j_mod, chunk, head, 64 v | 64 ones]
                vp_sb = vpool.tile([128, NTI, HPC, 128], bf16, tag="vp")
                nc.vector.memset(vp_sb[:, :, :, D:], 1.0)

                # ---------------- phase A: qkv gen + LN + transposes --------
                with tc.tile_pool(name="psA", bufs=2, space="PSUM") as psA, \
                     tc.tile_pool(name="psT", bufs=2, space="PSUM") as psT, \
                     tc.tile_pool(name="lnp", bufs=3) as lnp, \
                     tc.tile_pool(name="natp", bufs=4) as natp:
                    for ti in range(NTI):
                        pq = psA.tile([128, 6 * D], f32, tag="pq")
                        for ci in range(NCH):
                            nc.tensor.matmul(
                                pq,
                                lhsT=x_sb[:, ci, ti * 128:(ti + 1) * 128],
                                rhs=w_sb[:, ci, :],
                                start=(ci == 0),
                                stop=(ci == NCH - 1),
                            )
                        # LN instances: (hh, qk) -> psum col offset
                        offs = [(0, 0, 0), (0, 1, D), (1, 0, 3 * D), (1, 1, 4 * D)]
                        mv = lnp.tile([128, 4, 2], f32, tag="mv")
                        for idx, (hh, qk, off) in enumerate(offs):
                            st6 = lnp.tile([128, 6], f32, tag="st6")
                            nc.vector.bn_stats(st6, pq[:, off:off + D])
                            nc.vector.bn_aggr(mv[:, idx, :], st6)
                        stdv = lnp.tile([128, 4], f32, tag="stdv")
                        nc.scalar.activation(
                            out=stdv, in_=mv[:, :, 1],
                            func=mybir.ActivationFunctionType.Sqrt,
                            bias=eps_t, scale=1.0,
                        )
                        rstd = lnp.tile([128, 4], f32, tag="rstd")
                        nc.vector.reciprocal(out=rstd, in_=stdv)
                        # fold q scaling (D^-0.5) into rstd unless affine does it
                        if not affine_q:
                            nc.vector.tensor_scalar(
                                out=rstd[:, 0:4:2], in0=rstd[:, 0:4:2],
                                scalar1=QSCALE, scalar2=None,
                                op0=mybir.AluOpType.mult,
                            )
                        for idx, (hh, qk, off) in enumerate(offs):
                            affine = affine_q if qk == 0 else affine_k
                            nat = natp.tile([128, D], bf16, tag="nat")
                            if affine:
                                natf = natp.tile([128, D], f32, tag="natf")
                                nc.vector.tensor_scalar(
                                    out=natf, in0=pq[:, off:off + D],
                                    scalar1=mv[:, idx, 0:1],
                                    scalar2=rstd[:, idx:idx + 1],
                                    op0=mybir.AluOpType.subtract,
                                    op1=mybir.AluOpType.mult,
                                )
                                r = 0 if qk == 0 else 2
                                natf2 = natp.tile([128, D], f32, tag="natf2")
                                nc.vector.tensor_tensor(
                                    out=natf2, in0=natf, in1=aff_sb[:, r, :],
                                    op=mybir.AluOpType.mult,
                                )
                                nc.vector.tensor_tensor(
                                    out=nat, in0=natf2, in1=aff_sb[:, r + 1, :],
                                    op=mybir.AluOpType.add,
                                )
                            else:
                                nc.vector.tensor_scalar(
                                    out=nat, in0=pq[:, off:off + D],
                                    scalar1=mv[:, idx, 0:1],
                                    scalar2=rstd[:, idx:idx + 1],
                                    op0=mybir.AluOpType.subtract,
                                    op1=mybir.AluOpType.mult,
                                )
                            pt = psT.tile([D, 128], bf16, tag="pt")
                            nc.tensor.transpose(pt, nat, ident)
                            dst = qT_sb if qk == 0 else kT_sb
                            nc.vector.tensor_copy(
                                out=dst[hh * D:(hh + 1) * D, ti * 128:(ti + 1) * 128],
                                in_=pt,
                            )
                        for hh in range(HPC):
                            nc.vector.tensor_copy(
                                out=vp_sb[:, ti, hh, 0:D],
                                in_=pq[:, hh * 3 * D + 2 * D: hh * 3 * D + 3 * D],
                            )

                # ---------------- phase B: attention + projection -----------
                with tc.tile_pool(name="psS", bufs=2, space="PSUM") as psS, \
                     tc.tile_pool(name="psAV", bufs=2, space="PSUM") as psAV, \
                     tc.tile_pool(name="psP", bufs=1, space="PSUM") as psP, \
                     tc.tile_pool(name="esp", bufs=2) as esp, \
                     tc.tile_pool(name="otp", bufs=3) as otp, \
                     tc.tile_pool(name="outp", bufs=2) as outp, \
                     tc.tile_pool(name="mskp", bufs=2) as mskp:
                    for ti8 in range(NTI8):
                        i0 = ti8 * QI
                        oTs = []
                        for hh in range(HPC):
                            hs = slice(hh * D, (hh + 1) * D)
                            av = psAV.tile([128, QI], f32, tag="av")
                            for jq in range(4):
                                sT = psS.tile([128, 4, QI], f32, tag="sT")
                                for cj in range(4):
                                    j = jq * 4 + cj
                                    nc.tensor.matmul(
                                        sT[:, cj, :],
                                        lhsT=kT_sb[hs, j * 128:(j + 1) * 128],
                                        rhs=qT_sb[hs, i0:i0 + QI],
                                        start=True, stop=True,
                                    )
                                es = esp.tile([128, 4, QI], bf16, tag="es")
                                nc.scalar.activation(
                                    out=es, in_=sT,
                                    func=mybir.ActivationFunctionType.Exp,
                                )
                                if use_mask:
                                    msk = mskp.tile([128, 4, QI], bf16, tag="msk")
                                    for cj in range(4):
                                        j = jq * 4 + cj
                                        nc.sync.dma_start(
                                            out=msk[:, cj, :],
                                            in_=emaskT[j * 128:(j + 1) * 128,
                                                       i0:i0 + QI],
                                        )
                                    nc.vector.tensor_tensor(
                                        out=es, in0=es, in1=msk,
                                        op=mybir.AluOpType.mult,
                                    )
                                for cj in range(4):
                                    j = jq * 4 + cj
                                    nc.tensor.matmul(
                                        av,
                                        lhsT=vp_sb[:, j, hh, :],
                                        rhs=es[:, cj, :],
                                        start=(j == 0), stop=(j == NTI - 1),
                                    )
                            zr = otp.tile([D, QI], f32, tag="zr")
                            nc.vector.reciprocal(out=zr, in_=av[D:, :])
                            oT = otp.tile([D, QI], bf16, tag="oT")
                            nc.vector.tensor_tensor(
                                out=oT, in0=av[0:D, :], in1=zr,
                                op=mybir.AluOpType.mult,
                            )
                            oTs.append(oT)
                        for sub in range(QI // 128):
                            pp = psP.tile([128, DIM], f32, tag="pp")
                            for nh in range(2):
                                for hh in range(HPC):
                                    nc.tensor.matmul(
                                        pp[:, nh * 512:(nh + 1) * 512],
                                        lhsT=oTs[hh][:, sub * 128:(sub + 1) * 128],
                                        rhs=proj_sb[:, hh, nh * 512:(nh + 1) * 512],
                                        start=(hh == 0), stop=(hh == HPC - 1),
                                    )
                            osb = outp.tile([128, DIM], f32, tag="osb")
                            nc.vector.tensor_copy(out=osb, in_=pp)
                            r0 = i0 + sub * 128
                            nc.sync.dma_start(out=out_p[b, r0:r0 + 128, :], in_=osb)
    nc.compile()
    return nc


def _prep_inputs(inputs):
    x = np.ascontiguousarray(inputs["x"], dtype=F32)
    qkv_w = np.asarray(inputs["qkv_w"], dtype=F32)
    proj_w = np.asarray(inputs["proj_w"], dtype=F32)
    W_eff = qkv_w.copy()
    for i, (a, bm) in enumerate([("lora_Aq", "lora_Bq"), ("lora_Ak", "lora_Bk"),
                                 ("lora_Av", "lora_Bv")]):
        A = np.asarray(inputs[a], dtype=F32)
        Bm = np.asarray(inputs[bm], dtype=F32)
        W_eff[i * DIM:(i + 1) * DIM] += LORA_SCALE * (A @ Bm).T

    xT_all = np.ascontiguousarray(
        x.transpose(0, 2, 1).reshape(B, NCH, 128, N).astype(BF16))

    qn_w = np.asarray(inputs["qn_w"], F32); qn_b = np.asarray(inputs["qn_b"], F32)
    kn_w = np.asarray(inputs["kn_w"], F32); kn_b = np.asarray(inputs["kn_b"], F32)
    affine_q = not (np.all(qn_w == 1.0) and np.all(qn_b == 0.0))
    affine_k = not (np.all(kn_w == 1.0) and np.all(kn_b == 0.0))
    mask = np.asarray(inputs["attn_mask"], F32)
    use_mask = bool(np.any(mask))

    common = {"xT": xT_all}
    if affine_q or affine_k:
        aff = np.stack([
            np.broadcast_to(qn_w * QSCALE, (128, D)),
            np.broadcast_to(qn_b * QSCALE, (128, D)),
            np.broadcast_to(kn_w, (128, D)),
            np.broadcast_to(kn_b, (128, D)),
        ]).astype(F32)
        common["lnaff"] = np.ascontiguousarray(aff)
    if use_mask:
        common["emaskT"] = np.ascontiguousarray(
            np.exp(mask[0, 0].T).astype(BF16))

    in_maps = []
    for c in range(NCORES):
        h0 = c * HPC
        blocks = []
        for hh in range(HPC):
            h = h0 + hh
            for part in range(3):  # q, k, v
                blocks.append(W_eff[part * DIM + h * D: part * DIM + (h + 1) * D])
        Wlocal = np.concatenate(blocks, axis=0)          # [384, 1024]
        wT_c = np.ascontiguousarray(
            Wlocal.T.reshape(NCH, 128, 6 * D).astype(BF16))
        projT_c = np.ascontiguousarray(np.stack(
            [proj_w[:, (h0 + hh) * D:(h0 + hh + 1) * D].T for hh in range(HPC)],
            axis=1).astype(BF16))                        # [64, 2, 1024]
        m = dict(common)
        m["wT"] = wT_c
        m["projT"] = projT_c
        in_maps.append(m)
    return in_maps, (use_mask, affine_q, affine_k)


def _run(inputs, trace=False):
    in_maps, key = _prep_inputs(inputs)
    if key not in _prog_cache:
        _prog_cache[key] = _build_program(*key)
    nc = _prog_cache[key]
    res = run_bass_kernel_spmd(nc, in_maps, core_ids=list(range(NCORES)),
                               trace=trace)
    out = np.zeros((B, N, DIM), dtype=F32)
    for r in res.results:
        out += r["out_p"]
    out += np.asarray(inputs["proj_b"], F32)
    return out, res


def kernel(**inputs) -> np.ndarray:
    out, _ = _run(inputs)
    return out


# revision 12
# speedup vs baseline: 116.5710x; 116.5710x over previous
# Trainium2 Bass kernel for nn_CustomAttention (fused qkv + LoRA + per-head
# LayerNorm + softmax attention + output projection).
#
# Sharding: 16 heads split across 8 cores (2 heads/core), both batch elements
# on every core. Each core computes its heads' attention and its partial
# output projection (sum over its heads' columns); the host sums the 8
# partials and adds proj_b. LoRA is folded into the qkv weights on the host:
#   x@W.T + (x@A)@B*s == x@(W + s*(A@B).T).T
#
# Per-core layout choices (see comments in _build_program):
#  - scores are computed transposed (sT[j,i]) so softmax-normalized output
#    comes out in [d, i] layout, which is exactly the lhsT the projection
#    matmul needs -> no attention-weight transposes at all.
#  - exp without max subtraction (softmax is shift-invariant; post-LayerNorm
#    scores are bounded by ~|D|^0.5 so fp32 exp cannot overflow).
#  - the attention@v matmul uses stationary [v | ones]: output partitions
#    0-63 hold out^T, partitions 64-127 hold the softmax denominator
#    replicated, so normalization is a reciprocal + one multiply.
import numpy as np
import ml_dtypes

import concourse.bass as bass
import concourse.bacc as bacc
import concourse.mybir as mybir
from concourse.tile import TileContext
from concourse.masks import make_identity
from concourse.bass_utils import run_bass_kernel_spmd

BF16 = ml_dtypes.bfloat16
F32 = np.float32

B, N, DIM, H, R = 2, 2048, 1024, 16, 8
D = DIM // H              # 64
NCORES = 8
HPC = H // NCORES         # 2 heads per core
ALPHA = 8.0
LORA_SCALE = ALPHA / R
EPS = 1e-5
QSCALE = float(D) ** -0.5  # 0.125

NCH = DIM // 128          # 8 contraction chunks of 128
NTI = N // 128            # 16 row tiles of 128
NTI8 = N // 256           # 8 i-tiles of 256
QI = 256                  # query-tile width (free dim of score matmuls)

_prog_cache: dict = {}


def _build_program(use_mask: bool, affine_q: bool, affine_k: bool, repeat: int = 1):
    nc = bacc.Bacc("TRN2", target_bir_lowering=False)
    f32 = mybir.dt.float32
    bf16 = mybir.dt.bfloat16

    xT = nc.dram_tensor("xT", [B, NCH, 128, N], bf16, kind="ExternalInput")
    wT = nc.dram_tensor("wT", [NCH, 128, 6 * D], bf16, kind="ExternalInput")
    projT = nc.dram_tensor("projT", [D, HPC, DIM], bf16, kind="ExternalInput")
    out_p = nc.dram_tensor("out_p", [B, N, DIM], f32, kind="ExternalOutput")
    if affine_q or affine_k:
        # rows: 0=qw*scale 1=qb*scale 2=kw 3=kb, each broadcast to 128 parts
        lnaff = nc.dram_tensor("lnaff", [4, 128, D], f32, kind="ExternalInput")
    if use_mask:
        emaskT = nc.dram_tensor("emaskT", [N, N], bf16, kind="ExternalInput")

    with TileContext(nc) as tc:
        import contextlib
        with contextlib.ExitStack() as ctx:
            const = ctx.enter_context(tc.tile_pool(name="const", bufs=1))
            ident = const.tile([128, 128], bf16)
            make_identity(nc, ident)
            eps_t = const.tile([128, 1], f32)
            nc.vector.memset(eps_t, EPS)

            persist = ctx.enter_context(tc.tile_pool(name="persist", bufs=1))
            w_sb = persist.tile([128, NCH, 6 * D], bf16)
            nc.sync.dma_start(out=w_sb, in_=wT.rearrange("ci cm w -> cm ci w"))
            proj_sb = persist.tile([D, HPC, DIM], bf16)
            nc.sync.dma_start(out=proj_sb, in_=projT[:, :, :])
            if affine_q or affine_k:
                aff_sb = persist.tile([128, 4, D], f32)
                nc.sync.dma_start(out=aff_sb, in_=lnaff.rearrange("r p d -> p r d"))

            # per-b persistent activations (rewritten each b; Tile handles WAR)
            xpool = ctx.enter_context(tc.tile_pool(name="xpool", bufs=1))
            qkpool = ctx.enter_context(tc.tile_pool(name="qkpool", bufs=1))
            vpool = ctx.enter_context(tc.tile_pool(name="vpool", bufs=1))

            if repeat > 1:
                ctx.enter_context(tc.For_i(0, repeat, 1))
            for b in range(B):
                x_sb = xpool.tile([128, NCH, N], bf16, tag="x_sb")
                nc.sync.dma_start(out=x_sb, in_=xT[b].rearrange("ci cm n -> cm ci n"))
                # qT/kT: partitions 0-63 head0, 64-127 head1; free = n
                qT_sb = qkpool.tile([128, N], bf16, tag="qT")
                kT_sb = qkpool.tile([128, N], bf16, tag="kT")
                # vplus: [j_mod, chunk, head, 64 v | 64 ones]
                vp_sb = vpool.tile([128, NTI, HPC, 128], bf16, tag="vp")
                nc.vector.memset(vp_sb[:, :, :, D:], 1.0)

                # ---------------- phase A: qkv gen + LN + transposes --------
                with tc.tile_pool(name="psA", bufs=2, space="PSUM") as psA, \
                     tc.tile_pool(name="psT", bufs=2, space="PSUM") as psT, \
                     tc.tile_pool(name="lnp", bufs=3) as lnp, \
                     tc.tile_pool(name="natp", bufs=4) as natp:
                    for ti in range(NTI):
                        pq = psA.tile([128, 6 * D], f32, tag="pq")
                        for ci in range(NCH):
                            nc.tensor.matmul(
                                pq,
                                lhsT=x_sb[:, ci, ti * 128:(ti + 1) * 128],
                                rhs=w_sb[:, ci, :],
                                start=(ci == 0),
                                stop=(ci == NCH - 1),
                            )
                        # LN instances: (hh, qk) -> psum col offset
                        offs = [(0, 0, 0), (0, 1, D), (1, 0, 3 * D), (1, 1, 4 * D)]
                        mv = lnp.tile([128, 4, 2], f32, tag="mv")
                        for idx, (hh, qk, off) in enumerate(offs):
                            st6 = lnp.tile([128, 6], f32, tag="st6")
                            nc.vector.bn_stats(st6, pq[:, off:off + D])
                            nc.vector.bn_aggr(mv[:, idx, :], st6)
                        stdv = lnp.tile([128, 4], f32, tag="stdv")
                        nc.scalar.activation(
                            out=stdv, in_=mv[:, :, 1],
                            func=mybir.ActivationFunctionType.Sqrt,
                            bias=eps_t, scale=1.0,
                        )
                        rstd = lnp.tile([128, 4], f32, tag="rstd")
                        nc.vector.reciprocal(out=rstd, in_=stdv)
                        # fold q scaling (D^-0.5) into rstd unless affine does it
                        if not affine_q:
                            nc.vector.tensor_scalar(
                                out=rstd[:, 0:4:2], in0=rstd[:, 0:4:2],
                                scalar1=QSCALE, scalar2=None,
                                op0=mybir.AluOpType.mult,
                            )
                        for idx, (hh, qk, off) in enumerate(offs):
                            affine = affine_q if qk == 0 else affine_k
                            nat = natp.tile([128, D], bf16, tag="nat")
                            if affine:
                                natf = natp.tile([128, D], f32, tag="natf")
                                nc.vector.tensor_scalar(
                                    out=natf, in0=pq[:, off:off + D],
                                    scalar1=mv[:, idx, 0:1],
                                    scalar2=rstd[:, idx:idx + 1],
                                    op0=mybir.AluOpType.subtract,
                                    op1=mybir.AluOpType.mult,
                                )
                                r = 0 if qk == 0 else 2
                                natf2 = natp.tile([128, D], f32, tag="natf2")
                                nc.vector.tensor_tensor(
                                    out=natf2, in0=natf, in1=aff_sb[:, r, :],
                                    op=mybir.AluOpType.mult,
                                )
                                nc.vector.tensor_tensor(
                                    out=nat, in0=natf2, in1=aff_sb[:, r + 1, :],
                                    op=mybir.AluOpType.add,
                                )
                            else:
                                nc.vector.tensor_scalar(
                                    out=nat, in0=pq[:, off:off + D],
                                    scalar1=mv[:, idx, 0:1],
                                    scalar2=rstd[:, idx:idx + 1],
                                    op0=mybir.AluOpType.subtract,
                                    op1=mybir.AluOpType.mult,
                                )
                            pt = psT.tile([D, 128], bf16, tag="pt")
                            nc.tensor.transpose(pt, nat, ident)
                            dst = qT_sb if qk == 0 else kT_sb
                            nc.vector.tensor_copy(
                                out=dst[hh * D:(hh + 1) * D, ti * 128:(ti + 1) * 128],
                                in_=pt,
                            )
                        for hh in range(HPC):
                            nc.vector.tensor_copy(
                                out=vp_sb[:, ti, hh, 0:D],
                                in_=pq[:, hh * 3 * D + 2 * D: hh * 3 * D + 3 * D],
                            )

                # ---------------- phase B: attention + projection -----------
                with tc.tile_pool(name="psS", bufs=2, space="PSUM") as psS, \
                     tc.tile_pool(name="psAV", bufs=2, space="PSUM") as psAV, \
                     tc.tile_pool(name="psP", bufs=1, space="PSUM") as psP, \
                     tc.tile_pool(name="esp", bufs=2) as esp, \
                     tc.tile_pool(name="otp", bufs=3) as otp, \
                     tc.tile_pool(name="outp", bufs=2) as outp, \
                     tc.tile_pool(name="mskp", bufs=2) as mskp:
                    for ti8 in range(NTI8):
                        i0 = ti8 * QI
                        oTs = []
                        for hh in range(HPC):
                            hs = slice(hh * D, (hh + 1) * D)
                            av = psAV.tile([128, QI], f32, tag="av")
                            for jq in range(4):
                                sT = psS.tile([128, 4, QI], f32, tag="sT")
                                for cj in range(4):
                                    j = jq * 4 + cj
                                    nc.tensor.matmul(
                                        sT[:, cj, :],
                                        lhsT=kT_sb[hs, j * 128:(j + 1) * 128],
                                        rhs=qT_sb[hs, i0:i0 + QI],
                                        start=True, stop=True,
                                    )
                                es = esp.tile([128, 4, QI], bf16, tag="es")
                                nc.scalar.activation(
                                    out=es, in_=sT,
                                    func=mybir.ActivationFunctionType.Exp,
                                )
                                if use_mask:
                                    msk = mskp.tile([128, 4, QI], bf16, tag="msk")
                                    for cj in range(4):
                                        j = jq * 4 + cj
                                        nc.sync.dma_start(
                                            out=msk[:, cj, :],
                                            in_=emaskT[j * 128:(j + 1) * 128,
                                                       i0:i0 + QI],
                                        )
                                    nc.vector.tensor_tensor(
                                        out=es, in0=es, in1=msk,
                                        op=mybir.AluOpType.mult,
                                    )
                                for cj in range(4):
                                    j = jq * 4 + cj
                                    nc.tensor.matmul(
                                        av,
                                        lhsT=vp_sb[:, j, hh, :],
                                        rhs=es[:, cj, :],
                                        start=(j == 0), stop=(j == NTI - 1),
                                    )
                            zr = otp.tile([D, QI], f32, tag="zr")
                            nc.vector.reciprocal(out=zr, in_=av[D:, :])
                            oT = otp.tile([D, QI], bf16, tag="oT")
                            nc.vector.tensor_tensor(
                                out=oT, in0=av[0:D, :], in1=zr,
                                op=mybir.AluOpType.mult,
                            )
                            oTs.append(oT)
                        for sub in range(QI // 128):
                            pp = psP.tile([128, DIM], f32, tag="pp")
                            for nh in range(2):
                                for hh in range(HPC):
                                    nc.tensor.matmul(
                                        pp[:, nh * 512:(nh + 1) * 512],
                                        lhsT=oTs[hh][:, sub * 128:(sub + 1) * 128],
                                        rhs=proj_sb[:, hh, nh * 512:(nh + 1) * 512],
                                        start=(hh == 0), stop=(hh == HPC - 1),
                                    )
                            osb = outp.tile([128, DIM], f32, tag="osb")
                            nc.vector.tensor_copy(out=osb, in_=pp)
                            r0 = i0 + sub * 128
                            nc.sync.dma_start(out=out_p[b, r0:r0 + 128, :], in_=osb)
    nc.compile()
    return nc


def _prep_inputs(inputs):
    x = np.ascontiguousarray(inputs["x"], dtype=F32)
    qkv_w = np.asarray(inputs["qkv_w"], dtype=F32)
    proj_w = np.asarray(inputs["proj_w"], dtype=F32)
    W_eff = qkv_w.copy()
    for i, (a, bm) in enumerate([("lora_Aq", "lora_Bq"), ("lora_Ak", "lora_Bk"),
                                 ("lora_Av", "lora_Bv")]):
        A = np.asarray(inputs[a], dtype=F32)
        Bm = np.asarray(inputs[bm], dtype=F32)
        W_eff[i * DIM:(i + 1) * DIM] += LORA_SCALE * (A @ Bm).T

    xT_all = np.ascontiguousarray(
        x.transpose(0, 2, 1).reshape(B, NCH, 128, N).astype(BF16))

    qn_w = np.asarray(inputs["qn_w"], F32); qn_b = np.asarray(inputs["qn_b"], F32)
    kn_w = np.asarray(inputs["kn_w"], F32); kn_b = np.asarray(inputs["kn_b"], F32)
    affine_q = not (np.all(qn_w == 1.0) and np.all(qn_b == 0.0))
    affine_k = not (np.all(kn_w == 1.0) and np.all(kn_b == 0.0))
    mask = np.asarray(inputs["attn_mask"], F32)
    use_mask = bool(np.any(mask))

    common = {"xT": xT_all}
    if affine_q or affine_k:
        aff = np.stack([
            np.broadcast_to(qn_w * QSCALE, (128, D)),
            np.broadcast_to(qn_b * QSCALE, (128, D)),
            np.broadcast_to(kn_w, (128, D)),
            np.broadcast_to(kn_b, (128, D)),
        ]).astype(F32)
        common["lnaff"] = np.ascontiguousarray(aff)
    if use_mask:
        common["emaskT"] = np.ascontiguousarray(
            np.exp(mask[0, 0].T).astype(BF16))

    in_maps = []
    for c in range(NCORES):
        h0 = c * HPC
        blocks = []
        for hh in range(HPC):
            h = h0 + hh
            for part in range(3):  # q, k, v
                blocks.append(W_eff[part * DIM + h * D: part * DIM + (h + 1) * D])
        Wlocal = np.concatenate(blocks, axis=0)          # [384, 1024]
        wT_c = np.ascontiguousarray(
            Wlocal.T.reshape(NCH, 128, 6 * D).astype(BF16))
        projT_c = np.ascontiguousarray(np.stack(
            [proj_w[:, (h0 + hh) * D:(h0 + hh + 1) * D].T for hh in range(HPC)],
            axis=1).astype(BF16))                        # [64, 2, 1024]
        m = dict(common)
        m["wT"] = wT_c
        m["projT"] = projT_c
        in_maps.append(m)
    return in_maps, (use_mask, affine_q, affine_k)


def _run(inputs, trace=False):
    in_maps, key = _prep_inputs(inputs)
    if key not in _prog_cache:
        _prog_cache[key] = _build_program(*key)
    nc = _prog_cache[key]
    res = run_bass_kernel_spmd(nc, in_maps, core_ids=list(range(NCORES)),
                               trace=trace)
    out = np.zeros((B, N, DIM), dtype=F32)
    for r in res.results:
        out += r["out_p"]
    out += np.asarray(inputs["proj_b"], F32)
    return out, res


def kernel(**inputs) -> np.ndarray:
    out, _ = _run(inputs)
    return out


# revision 33
# speedup vs baseline: 148.4298x; 1.2733x over previous
# Trainium2 Bass kernel for nn_CustomAttention (fused qkv + LoRA + per-head
# LayerNorm + softmax attention + output projection).
#
# Sharding: 16 heads split across 8 cores (2 heads/core), both batch elements
# on every core. Each core computes its heads' attention and its partial
# output projection (sum over its heads' columns); the host sums the 8
# partials and adds proj_b. LoRA is folded into the qkv weights on the host:
#   x@W.T + (x@A)@B*s == x@(W + s*(A@B).T).T
#
# Per-core layout choices (see comments in _build_program):
#  - scores are computed transposed (sT[j,i]) so softmax-normalized output
#    comes out in [d, i] layout, which is exactly the lhsT the projection
#    matmul needs -> no attention-weight transposes at all.
#  - exp without max subtraction (softmax is shift-invariant; post-LayerNorm
#    scores are bounded by ~|D|^0.5 so fp32 exp cannot overflow).
#  - the attention@v matmul uses stationary [v | ones]: output partitions
#    0-63 hold out^T, partitions 64-127 hold the softmax denominator
#    replicated, so normalization is a reciprocal + one multiply.
import numpy as np
import ml_dtypes

import concourse.bass as bass
import concourse.bacc as bacc
import concourse.mybir as mybir
from concourse.tile import TileContext
from concourse.masks import make_identity
from concourse.bass_utils import run_bass_kernel_spmd

BF16 = ml_dtypes.bfloat16
F32 = np.float32

B, N, DIM, H, R = 2, 2048, 1024, 16, 8
D = DIM // H              # 64
NCORES = 8
HPC = H // NCORES         # 2 heads per core
ALPHA = 8.0
LORA_SCALE = ALPHA / R
EPS = 1e-5
QSCALE = float(D) ** -0.5  # 0.125

NCH = DIM // 128          # 8 contraction chunks of 128
NTI = N // 128            # 16 row tiles of 128
NTI8 = N // 256           # 8 i-tiles of 256
QI = 256                  # query-tile width (free dim of score matmuls)

_prog_cache: dict = {}


def _build_program(use_mask: bool, affine_q: bool, affine_k: bool, repeat: int = 1,
                   phases: str = "AB"):
    nc = bacc.Bacc("TRN2", target_bir_lowering=False)
    f32 = mybir.dt.float32
    bf16 = mybir.dt.bfloat16

    # xT layout: [cm, b, ci, n] so each partition's load is one contiguous
    # 32KB run; out_p layout: [cm, b, ti, c] for 8KB-contiguous stores.
    xT = nc.dram_tensor("xT", [128, B, NCH, N], bf16, kind="ExternalInput")
    wT = nc.dram_tensor("wT", [NCH, 128, 6 * D], bf16, kind="ExternalInput")
    projT = nc.dram_tensor("projT", [D, HPC, DIM], bf16, kind="ExternalInput")
    out_p = nc.dram_tensor("out_p", [128, B, NTI, DIM], f32, kind="ExternalOutput")
    if affine_q or affine_k:
        # rows: 0=qw*scale 1=qb*scale 2=kw 3=kb, each broadcast to 128 parts
        lnaff = nc.dram_tensor("lnaff", [4, 128, D], f32, kind="ExternalInput")
    if use_mask:
        emaskT = nc.dram_tensor("emaskT", [N, N], bf16, kind="ExternalInput")

    with TileContext(nc) as tc:
        import contextlib
        with contextlib.ExitStack() as ctx:
            const = ctx.enter_context(tc.tile_pool(name="const", bufs=1))
            ident = const.tile([128, 128], bf16)
            make_identity(nc, ident)
            eps_t = const.tile([128, 1], f32)
            nc.vector.memset(eps_t, EPS)

            persist = ctx.enter_context(tc.tile_pool(name="persist", bufs=1))
            w_sb = persist.tile([128, NCH, 6 * D], bf16)
            nc.sync.dma_start(out=w_sb, in_=wT.rearrange("ci cm w -> cm ci w"))
            proj_sb = persist.tile([D, HPC, DIM], bf16)
            nc.sync.dma_start(out=proj_sb, in_=projT[:, :, :])
            if affine_q or affine_k:
                aff_sb = persist.tile([128, 4, D], f32)
                nc.sync.dma_start(out=aff_sb, in_=lnaff.rearrange("r p d -> p r d"))

            # per-b persistent activations (rewritten each b; Tile handles WAR)
            xpool = ctx.enter_context(tc.tile_pool(name="xpool", bufs=2))
            qkpool = ctx.enter_context(tc.tile_pool(name="qkpool", bufs=2))
            vpool = ctx.enter_context(tc.tile_pool(name="vpool", bufs=2))

            if repeat > 1:
                ctx.enter_context(tc.For_i(
                    0, repeat, 1,
                    hint_engines=(mybir.EngineType.PE, mybir.EngineType.SP,
                                  mybir.EngineType.Activation,
                                  mybir.EngineType.DVE, mybir.EngineType.Pool)))
            # hoist both batches' input loads so b=1 prefetches under b=0
            x_sbs = []
            for b in range(B):
                x_sb = xpool.tile([128, NCH, N], bf16, tag="x_sb")
                nc.sync.dma_start(out=x_sb[:, 0:NCH // 2, :],
                                  in_=xT[:, b, 0:NCH // 2, :])
                nc.sync.dma_start(out=x_sb[:, NCH // 2:, :],
                                  in_=xT[:, b, NCH // 2:, :])
                x_sbs.append(x_sb)
            for b in range(B):
                x_sb = x_sbs[b]
                # qT/kT: partitions 0-63 head0, 64-127 head1; free = n
                qT_sb = qkpool.tile([128, N], bf16, tag="qT")
                kT_sb = qkpool.tile([128, N], bf16, tag="kT")
                # vplus: [j_mod, chunk, head, 64 v | 64 ones]
                vp_sb = vpool.tile([128, NTI, HPC, 128], bf16, tag="vp")
                nc.vector.memset(vp_sb[:, :, :, D:], 1.0)
                if "A" not in phases:  # timing variant: fill A outputs
                    nc.vector.memset(qT_sb, 0.5)
                    nc.vector.memset(kT_sb, 0.5)
                    nc.vector.memset(vp_sb[:, :, :, :D], 0.5)

                # ---------------- phase A: qkv gen + LN + transposes --------
                # qkv staged to SBUF f32; LayerNorm stats batched across all
                # 16 row-tiles x 4 instances into a few large ops.
                if "A" not in phases:
                    pass
                else:
                 with tc.tile_pool(name="psA", bufs=2, space="PSUM") as psA, \
                     tc.tile_pool(name="psT", bufs=2, space="PSUM") as psT, \
                     tc.tile_pool(name="stg", bufs=1) as stg, \
                     tc.tile_pool(name="lnp", bufs=2) as lnp, \
                     tc.tile_pool(name="natp", bufs=8) as natp:
                    stage = stg.tile([128, NTI, 6 * D], f32, tag="stage")
                    sqs = stg.tile([128, NTI, 6 * D], f32, tag="sqs")
                    for ti in range(NTI):
                        pq = psA.tile([128, 6 * D], f32, tag="pq")
                        for ci in range(NCH):
                            nc.tensor.matmul(
                                pq,
                                lhsT=x_sb[:, ci, ti * 128:(ti + 1) * 128],
                                rhs=w_sb[:, ci, :],
                                start=(ci == 0),
                                stop=(ci == NCH - 1),
                            )
                        nc.scalar.copy(out=stage[:, ti, :], in_=pq)
                        nc.vector.tensor_tensor(
                            out=sqs[:, ti, :], in0=stage[:, ti, :],
                            in1=stage[:, ti, :], op=mybir.AluOpType.mult)
                    # batched stats, in ti-halves so stats(h0) overlaps
                    # qkv matmuls of h1: [128, HT, 6, D] -> [128, HT*6]
                    st6v = stage.rearrange("p t (i d) -> p t i d", d=D)
                    sq6v = sqs.rearrange("p t (i d) -> p t i d", d=D)
                    HT = NTI // 2
                    insts = [(0, 0, 0), (1, 1, 0), (3, 0, 1), (4, 1, 1)]
                    for half in range(2):
                     hsl = slice(half * HT, (half + 1) * HT)
                     mean = lnp.tile([128, HT, 6], f32, tag="mean")
                     nc.vector.tensor_reduce(
                        out=mean, in_=st6v[:, hsl], axis=mybir.AxisListType.X,
                        op=mybir.AluOpType.add)
                     nc.vector.tensor_scalar(
                        out=mean, in0=mean, scalar1=1.0 / D, scalar2=None,
                        op0=mybir.AluOpType.mult)
                     var = lnp.tile([128, HT, 6], f32, tag="var")
                     nc.vector.tensor_reduce(
                        out=var, in_=sq6v[:, hsl], axis=mybir.AxisListType.X,
                        op=mybir.AluOpType.add)
                     nc.vector.tensor_scalar(
                        out=var, in0=var, scalar1=1.0 / D, scalar2=None,
                        op0=mybir.AluOpType.mult)
                     m2 = lnp.tile([128, HT, 6], f32, tag="m2")
                     nc.vector.tensor_tensor(
                        out=m2, in0=mean, in1=mean, op=mybir.AluOpType.mult)
                     nc.vector.tensor_tensor(
                        out=var, in0=var, in1=m2, op=mybir.AluOpType.subtract)
                     rstd = lnp.tile([128, HT, 6], f32, tag="rstd")
                     nc.scalar.activation(
                        out=rstd, in_=var,
                        func=mybir.ActivationFunctionType.Sqrt,
                        bias=eps_t, scale=1.0)
                     nc.vector.reciprocal(out=rstd, in_=rstd)
                     if not affine_q:  # fold q scaling (D^-0.5) into rstd
                        nc.vector.tensor_scalar(
                            out=rstd[:, :, 0:6:3], in0=rstd[:, :, 0:6:3],
                            scalar1=QSCALE, scalar2=None,
                            op0=mybir.AluOpType.mult)
                     for tih in range(HT):
                        ti = half * HT + tih
                        pt = psT.tile([128, 2, 128], bf16, tag="pt")
                        for inst, qk, hh in insts:
                            affine = affine_q if qk == 0 else affine_k
                            nat = natp.tile([128, D], bf16, tag="nat")
                            if affine:
                                natf = natp.tile([128, D], f32, tag="natf")
                                nc.vector.tensor_scalar(
                                    out=natf, in0=st6v[:, ti, inst, :],
                                    scalar1=mean[:, tih, inst:inst + 1],
                                    scalar2=rstd[:, tih, inst:inst + 1],
                                    op0=mybir.AluOpType.subtract,
                                    op1=mybir.AluOpType.mult)
                                r = 0 if qk == 0 else 2
                                natf2 = natp.tile([128, D], f32, tag="natf2")
                                nc.vector.tensor_tensor(
                                    out=natf2, in0=natf, in1=aff_sb[:, r, :],
                                    op=mybir.AluOpType.mult)
                                nc.vector.tensor_tensor(
                                    out=nat, in0=natf2, in1=aff_sb[:, r + 1, :],
                                    op=mybir.AluOpType.add)
                            else:
                                nc.vector.tensor_scalar(
                                    out=nat, in0=st6v[:, ti, inst, :],
                                    scalar1=mean[:, tih, inst:inst + 1],
                                    scalar2=rstd[:, tih, inst:inst + 1],
                                    op0=mybir.AluOpType.subtract,
                                    op1=mybir.AluOpType.mult)
                            nc.tensor.transpose(
                                pt[hh * D:(hh + 1) * D, qk, :], nat, ident)
                        nc.scalar.copy(
                            out=qT_sb[:, ti * 128:(ti + 1) * 128], in_=pt[:, 0, :])
                        nc.scalar.copy(
                            out=kT_sb[:, ti * 128:(ti + 1) * 128], in_=pt[:, 1, :])
                        nc.gpsimd.tensor_copy(
                            out=vp_sb[:, ti, :, 0:D],
                            in_=stage.rearrange("p t (h x) -> p t h x", h=2)
                                [:, ti, :, 2 * D:3 * D])

                # ---------------- phase B: attention + projection -----------
                if "B" not in phases:
                    pass
                else:
                 with tc.tile_pool(name="psS", bufs=2, space="PSUM") as psS, \
                     tc.tile_pool(name="psAV", bufs=2, space="PSUM") as psAV, \
                     tc.tile_pool(name="psP", bufs=1, space="PSUM") as psP, \
                     tc.tile_pool(name="esp", bufs=2) as esp, \
                     tc.tile_pool(name="otp", bufs=3) as otp, \
                     tc.tile_pool(name="outp", bufs=2) as outp, \
                     tc.tile_pool(name="mskp", bufs=2) as mskp:
                    for ti8 in range(NTI8):
                        i0 = ti8 * QI
                        oTs = []
                        for hh in range(HPC):
                            hs = slice(hh * D, (hh + 1) * D)
                            av = psAV.tile([128, QI], f32, tag="av")
                            for jq in range(4):
                                sT = psS.tile([128, 4, QI], f32, tag="sT")
                                for cj in range(4):
                                    j = jq * 4 + cj
                                    nc.tensor.matmul(
                                        sT[:, cj, :],
                                        lhsT=kT_sb[hs, j * 128:(j + 1) * 128],
                                        rhs=qT_sb[hs, i0:i0 + QI],
                                        start=True, stop=True,
                                    )
                                es = esp.tile([128, 4, QI], bf16, tag="es")
                                nc.scalar.activation(
                                    out=es, in_=sT,
                                    func=mybir.ActivationFunctionType.Exp,
                                )
                                if use_mask:
                                    msk = mskp.tile([128, 4, QI], bf16, tag="msk")
                                    for cj in range(4):
                                        j = jq * 4 + cj
                                        nc.sync.dma_start(
                                            out=msk[:, cj, :],
                                            in_=emaskT[j * 128:(j + 1) * 128,
                                                       i0:i0 + QI],
                                        )
                                    nc.vector.tensor_tensor(
                                        out=es, in0=es, in1=msk,
                                        op=mybir.AluOpType.mult,
                                    )
                                for cj in range(4):
                                    j = jq * 4 + cj
                                    nc.tensor.matmul(
                                        av,
                                        lhsT=vp_sb[:, j, hh, :],
                                        rhs=es[:, cj, :],
                                        start=(j == 0), stop=(j == NTI - 1),
                                    )
                            zr = otp.tile([D, QI], f32, tag="zr")
                            nc.vector.reciprocal(out=zr, in_=av[D:, :])
                            oT = otp.tile([D, QI], bf16, tag="oT")
                            nc.vector.tensor_tensor(
                                out=oT, in0=av[0:D, :], in1=zr,
                                op=mybir.AluOpType.mult,
                            )
                            oTs.append(oT)
                        osb = outp.tile([128, QI // 128, DIM], f32, tag="osb")
                        for sub in range(QI // 128):
                            pp = psP.tile([128, DIM], f32, tag="pp")
                            for nh in range(2):
                                for hh in range(HPC):
                                    nc.tensor.matmul(
                                        pp[:, nh * 512:(nh + 1) * 512],
                                        lhsT=oTs[hh][:, sub * 128:(sub + 1) * 128],
                                        rhs=proj_sb[:, hh, nh * 512:(nh + 1) * 512],
                                        start=(hh == 0), stop=(hh == HPC - 1),
                                    )
                            nc.vector.tensor_copy(out=osb[:, sub, :], in_=pp)
                        ti0 = ti8 * (QI // 128)
                        nc.scalar.dma_start(
                            out=out_p[:, b, ti0:ti0 + QI // 128, :], in_=osb)
    nc.compile()
    return nc


def _prep_inputs(inputs):
    x = np.ascontiguousarray(inputs["x"], dtype=F32)
    qkv_w = np.asarray(inputs["qkv_w"], dtype=F32)
    proj_w = np.asarray(inputs["proj_w"], dtype=F32)
    W_eff = qkv_w.copy()
    for i, (a, bm) in enumerate([("lora_Aq", "lora_Bq"), ("lora_Ak", "lora_Bk"),
                                 ("lora_Av", "lora_Bv")]):
        A = np.asarray(inputs[a], dtype=F32)
        Bm = np.asarray(inputs[bm], dtype=F32)
        W_eff[i * DIM:(i + 1) * DIM] += LORA_SCALE * (A @ Bm).T

    # [cm, b, ci, n] with cm = c % 128, ci = c // 128
    xT_all = np.ascontiguousarray(
        x.transpose(2, 0, 1).reshape(NCH, 128, B, N)
        .transpose(1, 2, 0, 3).astype(BF16))

    qn_w = np.asarray(inputs["qn_w"], F32); qn_b = np.asarray(inputs["qn_b"], F32)
    kn_w = np.asarray(inputs["kn_w"], F32); kn_b = np.asarray(inputs["kn_b"], F32)
    affine_q = not (np.all(qn_w == 1.0) and np.all(qn_b == 0.0))
    affine_k = not (np.all(kn_w == 1.0) and np.all(kn_b == 0.0))
    mask = np.asarray(inputs["attn_mask"], F32)
    use_mask = bool(np.any(mask))

    common = {"xT": xT_all}
    if affine_q or affine_k:
        aff = np.stack([
            np.broadcast_to(qn_w * QSCALE, (128, D)),
            np.broadcast_to(qn_b * QSCALE, (128, D)),
            np.broadcast_to(kn_w, (128, D)),
            np.broadcast_to(kn_b, (128, D)),
        ]).astype(F32)
        common["lnaff"] = np.ascontiguousarray(aff)
    if use_mask:
        common["emaskT"] = np.ascontiguousarray(
            np.exp(mask[0, 0].T).astype(BF16))

    in_maps = []
    for c in range(NCORES):
        h0 = c * HPC
        blocks = []
        for hh in range(HPC):
            h = h0 + hh
            for part in range(3):  # q, k, v
                blocks.append(W_eff[part * DIM + h * D: part * DIM + (h + 1) * D])
        Wlocal = np.concatenate(blocks, axis=0)          # [384, 1024]
        wT_c = np.ascontiguousarray(
            Wlocal.T.reshape(NCH, 128, 6 * D).astype(BF16))
        projT_c = np.ascontiguousarray(np.stack(
            [proj_w[:, (h0 + hh) * D:(h0 + hh + 1) * D].T for hh in range(HPC)],
            axis=1).astype(BF16))                        # [64, 2, 1024]
        m = dict(common)
        m["wT"] = wT_c
        m["projT"] = projT_c
        in_maps.append(m)
    return in_maps, (use_mask, affine_q, affine_k)


def _run(inputs, trace=False):
    in_maps, key = _prep_inputs(inputs)
    if key not in _prog_cache:
        _prog_cache[key] = _build_program(*key)
    nc = _prog_cache[key]
    res = run_bass_kernel_spmd(nc, in_maps, core_ids=list(range(NCORES)),
                               trace=trace)
    acc = np.zeros((128, B, NTI, DIM), dtype=F32)
    for r in res.results:
        acc += r["out_p"]
    # [cm, b, ti, c] -> [b, ti*128+cm, c]
    out = np.ascontiguousarray(acc.transpose(1, 2, 0, 3).reshape(B, N, DIM))
    out += np.asarray(inputs["proj_b"], F32)
    return out, res


def kernel(**inputs) -> np.ndarray:
    out, _ = _run(inputs)
    return out
